# revision 44
# baseline (speedup 1.0000x reference)
"""Trainium2 Bass kernel for nn_AdaptiveTokenFilter.

Reference computation (per batch row of S tokens):
  h = relu(x @ W1 + b1); logits = (h @ W2 + b2)[..., 0]
  expected_k = sum(sigmoid(logits)); k = max(int(expected_k), 32)
  mask = top-k(logits) one-hot; filtered = x * mask

Strategy: data-parallel over batch across 8 cores (2 batch rows per core),
no collectives. Per core:
  - GEMM1 in float32r (full-rate PE: 1 cycle/row at N>=256, e8m11 operand
    rounding), x transposed on-chip via fp32 PE transpose mode; GEMM2 in
    native fp32 (exact) to keep top-k boundary flips minimal.
  - Logits are kept in two layouts, built on the fly: a [1, S] row, and a
    [128, S/128] token-partition form (two extra PE transposes per macro
    tile) used for partition-parallel counting and for the mask columns.
  - Per-row top-k as a binary search for the count threshold. The k
    criterion is count > max(expected_k - 1, 31.5), exact for integer
    counts. The last row's search (an exposed tail) uses the partition-
    parallel count with a ones-matrix PE reduction; earlier rows use a
    DVE-only row count that overlaps the next row's GEMM.
  - filtered = x * mask re-streams x and scales each token row by its 0/1
    mask value. The masking pass for row b-1 is interleaved into row b's
    GEMM stream so its traffic hides under compute; the last row's pass
    alternates between both HWDGE queue families to double tail DMA
    bandwidth. Phase-2 emission is deferred into the next row's GEMM so
    the in-order DVE queue never blocks the next row's xT copies.
"""
import numpy as np

import concourse.bacc as bacc
import concourse.mybir as mybir
import concourse.tile as tile
from concourse.bass_utils import run_bass_kernel_spmd
from concourse.masks import make_identity
from concourse.bass import _add_dep_helper

F32 = mybir.dt.float32
F32R = mybir.dt.float32r
U32 = mybir.dt.uint32
ALU = mybir.AluOpType
ACT = mybir.ActivationFunctionType

N_CORES = 8


def build(B_PER_CORE=2, S=2048, D=2048, H=2048, N_TILE=256, N_ITERS=21,
          MIN_K=32.0):
    P = 128
    DC = D // P            # d-chunks
    HC = H // P            # h-chunks
    M_TILES = S // N_TILE  # macro token tiles per batch row
    SUBS = N_TILE // P     # 128-token subtiles per macro tile
    X_TILES = S // P       # 128-token tiles per batch row (masking pass)
    HALF = min(1024, D)    # stage tiles are [128, HALF]
    DHALVES = D // HALF
    TOK = B_PER_CORE * S
    LC = S // P            # logit columns in token-partition layout

    nc = bacc.Bacc("TRN2", target_bir_lowering=False, debug=False,
                   num_swdge_queues=4)

    x_d = nc.dram_tensor("x", [TOK, D], F32, kind="ExternalInput")
    w1_d = nc.dram_tensor("W1", [D, H], F32, kind="ExternalInput")
    b1_d = nc.dram_tensor("b1", [H], F32, kind="ExternalInput")
    w2_d = nc.dram_tensor("W2", [H, 1], F32, kind="ExternalInput")
    b2_d = nc.dram_tensor("b2", [1, 1], F32, kind="ExternalInput")

    filt_d = nc.dram_tensor("filtered", [TOK, D], F32, kind="ExternalOutput")
    mask_d = nc.dram_tensor("mask", [B_PER_CORE, S], F32, kind="ExternalOutput")
    ek_d = nc.dram_tensor("ek", [B_PER_CORE, 1], F32, kind="ExternalOutput")

    with tile.TileContext(nc) as tc:
        with (
            tc.tile_pool(name="const", bufs=1) as const_pool,
            tc.tile_pool(name="w1res", bufs=1) as w1_pool,
            tc.tile_pool(name="stage", bufs=5) as stage_pool,
            tc.tile_pool(name="stage3", bufs=3) as stage3_pool,
            tc.tile_pool(name="xt", bufs=1) as xt_pool,
            tc.tile_pool(name="ht", bufs=3) as ht_pool,
            tc.tile_pool(name="rows", bufs=1) as row_pool,
            tc.tile_pool(name="psum_t", bufs=2, space="PSUM") as psumt_pool,
            tc.tile_pool(name="psum", bufs=2, space="PSUM") as psum_pool,
            tc.tile_pool(name="psum_s", bufs=1, space="PSUM") as psums_pool,
            tc.tile_pool(name="dram", bufs=1, space="DRAM") as dram_pool,
        ):
            # ---------------- constants ----------------
            ident = const_pool.tile([P, P], F32, tag="ident")
            make_identity(nc, ident[:])
            ones_mat = const_pool.tile([P, P], F32, tag="ones_mat")
            nc.vector.memset(ones_mat[:], 1.0)
            ones_row = const_pool.tile([1, P], F32, tag="ones_row")
            nc.vector.memset(ones_row[:], 1.0)
            b2_sb = const_pool.tile([1, 1], F32, tag="b2")
            nc.sync.dma_start(b2_sb[:], b2_d[:])

            # b1 and W2 in [128, HC] chunk-column layout via PE transpose of
            # their natural [HC, 128] views (fast contiguous DMA).
            bw_nat = const_pool.tile([HC, P], F32, tag="bw_nat")
            nc.sync.dma_start(bw_nat[:], b1_d[:].rearrange("(c p) -> c p", p=P))
            bw_ps = psums_pool.tile([P, HC], F32, tag="ptl")
            nc.tensor.transpose(bw_ps[:], bw_nat[:], ident[:HC, :HC])
            b1_sb = const_pool.tile([P, HC], F32, tag="b1")
            nc.vector.tensor_copy(b1_sb[:], bw_ps[:])

            w2_nat = const_pool.tile([HC, P], F32, tag="w2_nat")
            nc.sync.dma_start(
                w2_nat[:], w2_d[:].rearrange("(c p) o -> c (p o)", p=P))
            w2_ps = psums_pool.tile([P, HC], F32, tag="ptl")
            nc.tensor.transpose(w2_ps[:], w2_nat[:], ident[:HC, :HC])
            w2_sb = const_pool.tile([P, HC], F32, tag="w2")
            nc.vector.tensor_copy(w2_sb[:], w2_ps[:])

            bounce = dram_pool.tile([B_PER_CORE, S], F32, tag="bounce")

            # ---------------- resident W1 (rounded to f32r) ----------------
            # SWDGE cast-DMA rounds fp32 -> f32r in flight, off the sync queues.
            w1_t = []
            for d in range(DC):
                wt = w1_pool.tile([P, H], F32R, tag=f"w1_{d}")
                hh = H // 2
                nc.gpsimd.dma_start(wt[:, :hh], w1_d[d * P:(d + 1) * P, :hh])
                nc.gpsimd.dma_start(wt[:, hh:], w1_d[d * P:(d + 1) * P, hh:])
                w1_t.append(wt)

            # mask pass for one batch row; `queues` alternates DMA engines
            def mask_pass_units(b, mc, alternate):
                for i in range(X_TILES):
                    tok0 = b * S + i * P
                    for hf in range(DHALVES):
                        eng = (nc.sync if (alternate and (i + hf) % 2 == 0)
                               else nc.scalar)
                        use_act = alternate and (i % 2 == 0)
                        def unit(i=i, hf=hf, tok0=tok0, mc=mc, eng=eng,
                                 use_act=use_act):
                            st = stage3_pool.tile([P, HALF], F32, tag="stage3")
                            eng.dma_start(
                                st[:],
                                x_d[tok0:tok0 + P, hf * HALF:(hf + 1) * HALF])
                            if use_act:
                                nc.scalar.activation(st[:], st[:], ACT.Copy,
                                                     scale=mc[:, i:i + 1])
                            else:
                                nc.vector.tensor_scalar(
                                    st[:], st[:], mc[:, i:i + 1], 0.0,
                                    op0=ALU.mult, op1=ALU.add)
                            eng.dma_start(
                                filt_d[tok0:tok0 + P, hf * HALF:(hf + 1) * HALF],
                                st[:])
                        yield unit

            # ---------------- phase 2 ----------------
            # Non-last rows: emitted in pieces woven into the next row's GEMM
            # stream (the row-form search is a serial DVE chain; explicit
            # ordering edges keep it behind each macro's xT copies so the
            # in-order DVE queue never starves the PE).
            def make_phase2_parts(b, logits):
                st = {}

                def head(after_inst):
                    scratch = row_pool.tile([1, S], F32, tag="rowscratch")
                    ek = row_pool.tile([1, 1], F32, tag=f"ek{b}")
                    first = nc.scalar.activation(scratch[:], logits[:],
                                                 ACT.Sigmoid, accum_out=ek[:])
                    if after_inst is not None:
                        _add_dep_helper(first.ins, after_inst.ins, sync=False,
                                        reason="phase2 after xT copies")
                    nc.scalar.dma_start(ek_d[b:b + 1, :], ek[:])
                    ekm1 = row_pool.tile([1, 1], F32, tag="ekm1")
                    nc.vector.tensor_scalar(ekm1[:], ek[:], -1.0, 31.5,
                                            op0=ALU.add, op1=ALU.max)
                    lo = row_pool.tile([1, 1], F32, tag="lo")
                    hi = row_pool.tile([1, 1], F32, tag="hi")
                    mid = row_pool.tile([1, 1], F32, tag="mid")
                    cnt = row_pool.tile([1, 1], F32, tag="cnt")
                    pred = row_pool.tile([1, 1], U32, tag="pred")
                    predn = row_pool.tile([1, 1], U32, tag="predn")
                    nc.vector.tensor_reduce(
                        hi[:], logits[:], axis=mybir.AxisListType.X, op=ALU.max)
                    nc.vector.tensor_reduce(
                        lo[:], logits[:], axis=mybir.AxisListType.X, op=ALU.min)
                    nc.vector.tensor_scalar(lo[:], lo[:], -1.0, 0.0,
                                            op0=ALU.add, op1=ALU.add)
                    st.update(scratch=scratch, ekm1=ekm1, lo=lo, hi=hi,
                              mid=mid, cnt=cnt, pred=pred, predn=predn)

                def chunk(n, after_inst):
                    lo, hi, mid, cnt = st["lo"], st["hi"], st["mid"], st["cnt"]
                    pred, predn, ekm1 = st["pred"], st["predn"], st["ekm1"]
                    for j in range(n):
                        first = nc.vector.tensor_tensor(mid[:], lo[:], hi[:],
                                                        op=ALU.add)
                        if j == 0 and after_inst is not None:
                            _add_dep_helper(first.ins, after_inst.ins,
                                            sync=False,
                                            reason="search after xT copies")
                        nc.vector.tensor_scalar(mid[:], mid[:], 0.5, 0.0,
                                                op0=ALU.mult, op1=ALU.add)
                        nc.vector.tensor_scalar(
                            st["scratch"][:], logits[:], mid[0:1, 0:1], 0.0,
                            op0=ALU.is_gt, op1=ALU.add, accum_out=cnt[:])
                        nc.vector.tensor_tensor(pred[:], cnt[:], ekm1[:],
                                                op=ALU.is_gt)
                        nc.vector.tensor_tensor(predn[:], cnt[:], ekm1[:],
                                                op=ALU.is_le)
                        nc.vector.copy_predicated(lo[:], pred[:], mid[:])
                        nc.vector.copy_predicated(hi[:], predn[:], mid[:])

                def finish():
                    scratch, lo = st["scratch"], st["lo"]
                    nc.vector.tensor_scalar(
                        scratch[:], logits[:], lo[0:1, 0:1], 0.0,
                        op0=ALU.is_gt, op1=ALU.add)
                    nc.scalar.dma_start(mask_d[b:b + 1, :], scratch[:])
                    nc.scalar.dma_start(bounce[b:b + 1, :], scratch[:])
                    mc = row_pool.tile([P, X_TILES], F32, tag=f"mc{b}")
                    nc.scalar.dma_start(
                        mc[:], bounce[b, :].rearrange("(i p) -> p i", p=P))
                    return mc

                return head, chunk, finish

            # Last row: single-shot phase 2 at the tail (PE idle there).
            def make_phase2(b, logits, lcol):
                def phase2():
                    scratch = row_pool.tile([1, S], F32, tag="rowscratch")
                    ek = row_pool.tile([1, 1], F32, tag=f"ek{b}")
                    nc.scalar.activation(scratch[:], logits[:], ACT.Sigmoid,
                                         accum_out=ek[:])
                    nc.scalar.dma_start(ek_d[b:b + 1, :], ek[:])
                    # criterion: cnt >= max(floor(ek),32) <=> cnt > max(ek-1,31.5)
                    ekm1 = row_pool.tile([1, 1], F32, tag="ekm1")
                    nc.vector.tensor_scalar(ekm1[:], ek[:], -1.0, 31.5,
                                            op0=ALU.add, op1=ALU.max)

                    pe_search = (b == B_PER_CORE - 1)
                    if pe_search:
                        # token-partition logits via a burst of thin PE
                        # transposes (deps long satisfied; no queue stalls)
                        for mm in range(M_TILES):
                            ptl = psums_pool.tile([P, SUBS], F32, tag="ptl")
                            for sub in range(SUBS):
                                nc.tensor.transpose(
                                    ptl[:, sub:sub + 1],
                                    logits[:, mm * N_TILE + sub * P:
                                           mm * N_TILE + (sub + 1) * P],
                                    ones_row[0:1, 0:1])
                            nc.vector.tensor_copy(
                                lcol[:, mm * SUBS:(mm + 1) * SUBS], ptl[:])
                        lo = row_pool.tile([P, 1], F32, tag="lo128")
                        hi = row_pool.tile([P, 1], F32, tag="hi128")
                        mid = row_pool.tile([P, 1], F32, tag="mid128")
                        pc = row_pool.tile([P, 1], F32, tag="pc")
                        cscr = row_pool.tile([P, LC], F32, tag="cscr")
                        pred = row_pool.tile([P, 1], U32, tag="pred128")
                        predn = row_pool.tile([P, 1], U32, tag="predn128")
                        ekm128 = row_pool.tile([P, 1], F32, tag="ekm128")
                        ek_ps = psums_pool.tile([P, 1], F32, tag="small")
                        nc.tensor.matmul(ek_ps[:], ones_row[:], ekm1[:],
                                         start=True, stop=True)
                        nc.vector.tensor_copy(ekm128[:], ek_ps[:])
                        nc.vector.memset(lo[:], -16.0)
                        nc.vector.memset(hi[:], 16.0)
                        for _ in range(N_ITERS):
                            nc.vector.tensor_tensor(mid[:], lo[:], hi[:], op=ALU.add)
                            nc.vector.tensor_scalar(mid[:], mid[:], 0.5, 0.0,
                                                    op0=ALU.mult, op1=ALU.add)
                            nc.vector.tensor_scalar(
                                cscr[:], lcol[:], mid[:], 0.0,
                                op0=ALU.is_gt, op1=ALU.add, accum_out=pc[:])
                            cp = psums_pool.tile([P, 1], F32, tag="small")
                            nc.tensor.matmul(cp[:], ones_mat[:], pc[:],
                                             start=True, stop=True)
                            nc.vector.tensor_tensor(pred[:], cp[:], ekm128[:],
                                                    op=ALU.is_gt)
                            nc.vector.tensor_tensor(predn[:], cp[:], ekm128[:],
                                                    op=ALU.is_le)
                            nc.vector.copy_predicated(lo[:], pred[:], mid[:])
                            nc.vector.copy_predicated(hi[:], predn[:], mid[:])
                        lo128 = lo
                    else:
                        lo = row_pool.tile([1, 1], F32, tag="lo")
                        hi = row_pool.tile([1, 1], F32, tag="hi")
                        mid = row_pool.tile([1, 1], F32, tag="mid")
                        cnt = row_pool.tile([1, 1], F32, tag="cnt")
                        pred = row_pool.tile([1, 1], U32, tag="pred")
                        predn = row_pool.tile([1, 1], U32, tag="predn")
                        nc.vector.tensor_reduce(
                            hi[:], logits[:], axis=mybir.AxisListType.X, op=ALU.max)
                        nc.vector.tensor_reduce(
                            lo[:], logits[:], axis=mybir.AxisListType.X, op=ALU.min)
                        nc.vector.tensor_scalar(lo[:], lo[:], -1.0, 0.0,
                                                op0=ALU.add, op1=ALU.add)
                        for _ in range(N_ITERS):
                            nc.vector.tensor_tensor(mid[:], lo[:], hi[:], op=ALU.add)
                            nc.vector.tensor_scalar(mid[:], mid[:], 0.5, 0.0,
                                                    op0=ALU.mult, op1=ALU.add)
                            nc.vector.tensor_scalar(
                                scratch[:], logits[:], mid[0:1, 0:1], 0.0,
                                op0=ALU.is_gt, op1=ALU.add, accum_out=cnt[:])
                            nc.vector.tensor_tensor(pred[:], cnt[:], ekm1[:],
                                                    op=ALU.is_gt)
                            nc.vector.tensor_tensor(predn[:], cnt[:], ekm1[:],
                                                    op=ALU.is_le)
                            nc.vector.copy_predicated(lo[:], pred[:], mid[:])
                            nc.vector.copy_predicated(hi[:], predn[:], mid[:])
                        lo128 = lo

                    # mask row output = logits > lo
                    nc.vector.tensor_scalar(
                        scratch[:], logits[:], lo128[0:1, 0:1], 0.0,
                        op0=ALU.is_gt, op1=ALU.add)
                    nc.scalar.dma_start(mask_d[b:b + 1, :], scratch[:])
                    # mask columns for the masking pass
                    mc = row_pool.tile([P, X_TILES], F32, tag=f"mc{b}")
                    if pe_search:
                        nc.vector.tensor_scalar(
                            mc[:], lcol[:], lo128[:], 0.0,
                            op0=ALU.is_gt, op1=ALU.add)
                    else:
                        nc.scalar.dma_start(bounce[b:b + 1, :], scratch[:])
                        nc.scalar.dma_start(
                            mc[:], bounce[b, :].rearrange("(i p) -> p i", p=P))
                    return mc
                return phase2

            # ---------------- per-batch-row pipeline ----------------
            deferred_parts = None  # (head, chunk, finish) of the previous row
            deferred_phase2 = None
            pending = []
            iters_left = 0
            CHUNK = (N_ITERS + 2) // 3

            for b in range(B_PER_CORE):
                logits = row_pool.tile([1, S], F32, tag=f"logits{b}")
                lcol = row_pool.tile([P, LC], F32, tag=f"lcol{b}")

                for m in range(M_TILES):
                    tok0 = b * S + m * N_TILE
                    # ---- load + transpose x macro tile -> xT[d] (f32r)
                    stg = [[None] * DHALVES for _ in range(SUBS)]
                    for sub in range(SUBS):
                        for hf in range(DHALVES):
                            st = stage_pool.tile([P, HALF], F32, tag="stage")
                            nc.sync.dma_start(
                                st[:],
                                x_d[tok0 + sub * P: tok0 + (sub + 1) * P,
                                    hf * HALF:(hf + 1) * HALF])
                            stg[sub][hf] = st
                    xts = []
                    for d in range(DC):
                        hf, off = divmod(d * P, HALF)
                        pt = psumt_pool.tile([P, N_TILE], F32, tag="pt")
                        for sub in range(SUBS):
                            nc.tensor.transpose(
                                pt[:, sub * P:(sub + 1) * P],
                                stg[sub][hf][:, off:off + P], ident[:])
                        xt = xt_pool.tile([P, N_TILE], F32R, tag=f"xt{d}")
                        last_xt = nc.vector.tensor_copy(xt[:], pt[:])
                        xts.append(xt)

                    # ---- GEMM1 (f32r) + relu-evict (ACT) + GEMM2 (fp32)
                    pl = psum_pool.tile([1, N_TILE], F32, tag="pl")
                    for h in range(HC):
                        ph = psum_pool.tile([P, N_TILE], F32, tag="ph")
                        for d in range(DC):
                            nc.tensor.matmul(
                                ph[:], w1_t[d][:, h * P:(h + 1) * P], xts[d][:],
                                start=(d == 0), stop=(d == DC - 1))
                        ht = ht_pool.tile([P, N_TILE], F32, tag="ht")
                        nc.scalar.activation(ht[:], ph[:], ACT.Relu,
                                             bias=b1_sb[:, h:h + 1])
                        nc.tensor.matmul(
                            pl[:], w2_sb[:, h:h + 1], ht[:],
                            start=(h == 0), stop=(h == HC - 1),
                            skip_group_check=True)
                    # logits row slice = psum + b2
                    nc.vector.tensor_scalar(
                        logits[:, m * N_TILE:(m + 1) * N_TILE], pl[0:1, :],
                        b2_sb[0:1, 0:1], 0.0, op0=ALU.add, op1=ALU.add)
                    # weave the previous row's phase 2 into this GEMM stream
                    if deferred_parts is not None:
                        p2_head, p2_chunk, p2_finish = deferred_parts
                        if m == 0:
                            p2_head(last_xt)
                            p2_chunk(CHUNK, None)
                            iters_left = N_ITERS - CHUNK
                        elif iters_left > 0:
                            take = min(CHUNK, iters_left)
                            p2_chunk(take, last_xt)
                            iters_left -= take
                            if iters_left == 0:
                                mc_prev = p2_finish()
                                pending = list(mask_pass_units(
                                    b - 1, mc_prev, False))
                                deferred_parts = None
                    elif pending:
                        # spread the previous row's masking pass over macros
                        for _ in range(7):
                            if pending:
                                pending.pop(0)()

                if deferred_parts is not None:
                    # small-config fallback: flush the previous row's phase 2
                    p2_head, p2_chunk, p2_finish = deferred_parts
                    if iters_left >= N_ITERS:
                        p2_head(None)
                    if iters_left > 0:
                        p2_chunk(iters_left, None)
                    mc_prev = p2_finish()
                    pending = list(mask_pass_units(b - 1, mc_prev, False))
                    deferred_parts = None
                while pending:
                    pending.pop(0)()
                if b < B_PER_CORE - 1:
                    deferred_parts = make_phase2_parts(b, logits)
                    iters_left = N_ITERS
                else:
                    deferred_phase2 = make_phase2(b, logits, lcol)

            # tail: last row's phase 2 + masking pass on both queue families
            mc_last = deferred_phase2()
            for unit in mask_pass_units(B_PER_CORE - 1, mc_last, True):
                unit()

    nc.compile()
    return nc


_FULL_NC = None
TRACE = False          # test harness sets True to capture exec_time_ns
LAST_EXEC_NS = None
LAST_RESULT = None


def kernel(token_embeddings, W1, b1, W2, b2):
    global _FULL_NC
    B, S, D = token_embeddings.shape
    H = W1.shape[1]
    assert (B, S, D, H) == (16, 2048, 2048, 2048), (B, S, D, H)
    bpc = B // N_CORES
    if _FULL_NC is None:
        _FULL_NC = build(B_PER_CORE=bpc, S=S, D=D, H=H)
    nc = _FULL_NC

    x = np.ascontiguousarray(np.asarray(token_embeddings, dtype=np.float32))
    W1 = np.ascontiguousarray(np.asarray(W1, dtype=np.float32))
    b1 = np.ascontiguousarray(np.asarray(b1, dtype=np.float32))
    W2 = np.ascontiguousarray(np.asarray(W2, dtype=np.float32))
    b2 = np.ascontiguousarray(np.asarray(b2, dtype=np.float32)).reshape(1, 1)

    in_maps = []
    for c in range(N_CORES):
        in_maps.append({
            "x": x[c * bpc:(c + 1) * bpc].reshape(bpc * S, D),
            "W1": W1, "b1": b1, "W2": W2, "b2": b2,
        })
    global LAST_EXEC_NS, LAST_RESULT
    res = run_bass_kernel_spmd(nc, in_maps, core_ids=list(range(N_CORES)),
                               trace=TRACE)
    LAST_EXEC_NS = res.exec_time_ns
    LAST_RESULT = res
    filtered = np.empty((B, S, D), np.float32)
    mask = np.empty((B, S), np.float32)
    ek = np.empty((B,), np.float32)
    for c, r in enumerate(res.results):
        filtered[c * bpc:(c + 1) * bpc] = r["filtered"].reshape(bpc, S, D)
        mask[c * bpc:(c + 1) * bpc] = r["mask"]
        ek[c * bpc:(c + 1) * bpc] = r["ek"].ravel()
    return filtered, mask, ek


# revision 45
# speedup vs baseline: 1.0291x; 1.0291x over previous
"""Trainium2 Bass kernel for nn_AdaptiveTokenFilter.

Reference computation (per batch row of S tokens):
  h = relu(x @ W1 + b1); logits = (h @ W2 + b2)[..., 0]
  expected_k = sum(sigmoid(logits)); k = max(int(expected_k), 32)
  mask = top-k(logits) one-hot; filtered = x * mask

Strategy: data-parallel over batch across 8 cores (2 batch rows per core),
no collectives. Per core:
  - GEMM1 in float32r (full-rate PE: 1 cycle/row at N>=256, e8m11 operand
    rounding), x transposed on-chip via fp32 PE transpose mode; GEMM2 in
    native fp32 (exact) to keep top-k boundary flips minimal.
  - Logits are kept in two layouts, built on the fly: a [1, S] row, and a
    [128, S/128] token-partition form (two extra PE transposes per macro
    tile) used for partition-parallel counting and for the mask columns.
  - Per-row top-k as a binary search for the count threshold. The k
    criterion is count > max(expected_k - 1, 31.5), exact for integer
    counts. The last row's search (an exposed tail) uses the partition-
    parallel count with a ones-matrix PE reduction; earlier rows use a
    DVE-only row count that overlaps the next row's GEMM.
  - filtered = x * mask re-streams x and scales each token row by its 0/1
    mask value. The masking pass for row b-1 is interleaved into row b's
    GEMM stream so its traffic hides under compute; the last row's pass
    alternates between both HWDGE queue families to double tail DMA
    bandwidth. Phase-2 emission is deferred into the next row's GEMM so
    the in-order DVE queue never blocks the next row's xT copies.
"""
import numpy as np

import concourse.bacc as bacc
import concourse.mybir as mybir
import concourse.tile as tile
from concourse.bass_utils import run_bass_kernel_spmd
from concourse.masks import make_identity
from concourse.bass import _add_dep_helper

F32 = mybir.dt.float32
F32R = mybir.dt.float32r
U32 = mybir.dt.uint32
ALU = mybir.AluOpType
ACT = mybir.ActivationFunctionType

N_CORES = 8


def build(B_PER_CORE=2, S=2048, D=2048, H=2048, N_TILE=256, N_ITERS=21,
          MIN_K=32.0):
    P = 128
    DC = D // P            # d-chunks
    HC = H // P            # h-chunks
    M_TILES = S // N_TILE  # macro token tiles per batch row
    SUBS = N_TILE // P     # 128-token subtiles per macro tile
    X_TILES = S // P       # 128-token tiles per batch row (masking pass)
    HALF = min(1024, D)    # stage tiles are [128, HALF]
    DHALVES = D // HALF
    TOK = B_PER_CORE * S
    LC = S // P            # logit columns in token-partition layout

    nc = bacc.Bacc("TRN2", target_bir_lowering=False, debug=False,
                   num_swdge_queues=4)

    x_d = nc.dram_tensor("x", [TOK, D], F32, kind="ExternalInput")
    w1_d = nc.dram_tensor("W1", [D, H], F32, kind="ExternalInput")
    b1_d = nc.dram_tensor("b1", [H], F32, kind="ExternalInput")
    w2_d = nc.dram_tensor("W2", [H, 1], F32, kind="ExternalInput")
    b2_d = nc.dram_tensor("b2", [1, 1], F32, kind="ExternalInput")

    filt_d = nc.dram_tensor("filtered", [TOK, D], F32, kind="ExternalOutput")
    mask_d = nc.dram_tensor("mask", [B_PER_CORE, S], F32, kind="ExternalOutput")
    ek_d = nc.dram_tensor("ek", [B_PER_CORE, 1], F32, kind="ExternalOutput")

    with tile.TileContext(nc) as tc:
        with (
            tc.tile_pool(name="const", bufs=1) as const_pool,
            tc.tile_pool(name="w1res", bufs=1) as w1_pool,
            tc.tile_pool(name="stage", bufs=5) as stage_pool,
            tc.tile_pool(name="stage3", bufs=3) as stage3_pool,
            tc.tile_pool(name="xt", bufs=1) as xt_pool,
            tc.tile_pool(name="ht", bufs=3) as ht_pool,
            tc.tile_pool(name="rows", bufs=1) as row_pool,
            tc.tile_pool(name="psum_t", bufs=3, space="PSUM") as psumt_pool,
            tc.tile_pool(name="psum_l", bufs=1, space="PSUM") as psuml_pool,
            tc.tile_pool(name="psum", bufs=2, space="PSUM") as psum_pool,
            tc.tile_pool(name="psum_s", bufs=1, space="PSUM") as psums_pool,
            tc.tile_pool(name="dram", bufs=1, space="DRAM") as dram_pool,
        ):
            # ---------------- constants ----------------
            ident = const_pool.tile([P, P], F32, tag="ident")
            make_identity(nc, ident[:])
            ones_mat = const_pool.tile([P, P], F32, tag="ones_mat")
            nc.vector.memset(ones_mat[:], 1.0)
            ones_row = const_pool.tile([1, P], F32, tag="ones_row")
            nc.vector.memset(ones_row[:], 1.0)
            b2_sb = const_pool.tile([1, 1], F32, tag="b2")
            nc.sync.dma_start(b2_sb[:], b2_d[:])

            # b1 and W2 in [128, HC] chunk-column layout via PE transpose of
            # their natural [HC, 128] views (fast contiguous DMA).
            bw_nat = const_pool.tile([HC, P], F32, tag="bw_nat")
            nc.sync.dma_start(bw_nat[:], b1_d[:].rearrange("(c p) -> c p", p=P))
            bw_ps = psums_pool.tile([P, HC], F32, tag="ptl")
            nc.tensor.transpose(bw_ps[:], bw_nat[:], ident[:HC, :HC])
            b1_sb = const_pool.tile([P, HC], F32, tag="b1")
            nc.vector.tensor_copy(b1_sb[:], bw_ps[:])

            w2_nat = const_pool.tile([HC, P], F32, tag="w2_nat")
            nc.sync.dma_start(
                w2_nat[:], w2_d[:].rearrange("(c p) o -> c (p o)", p=P))
            w2_ps = psums_pool.tile([P, HC], F32, tag="ptl")
            nc.tensor.transpose(w2_ps[:], w2_nat[:], ident[:HC, :HC])
            w2_sb = const_pool.tile([P, HC], F32, tag="w2")
            nc.vector.tensor_copy(w2_sb[:], w2_ps[:])

            bounce = dram_pool.tile([B_PER_CORE, S], F32, tag="bounce")

            # ---------------- resident W1 (rounded to f32r) ----------------
            # SWDGE cast-DMA rounds fp32 -> f32r in flight, off the sync queues.
            w1_t = []
            for d in range(DC):
                wt = w1_pool.tile([P, H], F32R, tag=f"w1_{d}")
                hh = H // 2
                nc.gpsimd.dma_start(wt[:, :hh], w1_d[d * P:(d + 1) * P, :hh])
                nc.gpsimd.dma_start(wt[:, hh:], w1_d[d * P:(d + 1) * P, hh:])
                w1_t.append(wt)

            # mask pass for one batch row; `queues` alternates DMA engines
            def mask_pass_units(b, mc, alternate):
                for i in range(X_TILES):
                    tok0 = b * S + i * P
                    for hf in range(DHALVES):
                        eng = (nc.sync if (alternate and (i + hf) % 2 == 0)
                               else nc.scalar)
                        use_act = alternate and (i % 2 == 0)
                        def unit(i=i, hf=hf, tok0=tok0, mc=mc, eng=eng,
                                 use_act=use_act):
                            st = stage3_pool.tile([P, HALF], F32, tag="stage3")
                            eng.dma_start(
                                st[:],
                                x_d[tok0:tok0 + P, hf * HALF:(hf + 1) * HALF])
                            if use_act:
                                nc.scalar.activation(st[:], st[:], ACT.Copy,
                                                     scale=mc[:, i:i + 1])
                            else:
                                nc.vector.tensor_scalar(
                                    st[:], st[:], mc[:, i:i + 1], 0.0,
                                    op0=ALU.mult, op1=ALU.add)
                            eng.dma_start(
                                filt_d[tok0:tok0 + P, hf * HALF:(hf + 1) * HALF],
                                st[:])
                        yield unit

            # ---------------- phase 2 ----------------
            # Non-last rows: emitted in pieces woven into the next row's GEMM
            # stream (the row-form search is a serial DVE chain; explicit
            # ordering edges keep it behind each macro's xT copies so the
            # in-order DVE queue never starves the PE).
            def make_phase2_parts(b, logits):
                st = {}

                def head(after_inst):
                    scratch = row_pool.tile([1, S], F32, tag="rowscratch")
                    ek = row_pool.tile([1, 1], F32, tag=f"ek{b}")
                    first = nc.scalar.activation(scratch[:], logits[:],
                                                 ACT.Sigmoid, accum_out=ek[:])
                    if after_inst is not None:
                        _add_dep_helper(first.ins, after_inst.ins, sync=False,
                                        reason="phase2 after xT copies")
                    nc.scalar.dma_start(ek_d[b:b + 1, :], ek[:])
                    ekm1 = row_pool.tile([1, 1], F32, tag="ekm1")
                    nc.vector.tensor_scalar(ekm1[:], ek[:], -1.0, 31.5,
                                            op0=ALU.add, op1=ALU.max)
                    lo = row_pool.tile([1, 1], F32, tag="lo")
                    hi = row_pool.tile([1, 1], F32, tag="hi")
                    mid = row_pool.tile([1, 1], F32, tag="mid")
                    cnt = row_pool.tile([1, 1], F32, tag="cnt")
                    pred = row_pool.tile([1, 1], U32, tag="pred")
                    predn = row_pool.tile([1, 1], U32, tag="predn")
                    nc.vector.tensor_reduce(
                        hi[:], logits[:], axis=mybir.AxisListType.X, op=ALU.max)
                    nc.vector.tensor_reduce(
                        lo[:], logits[:], axis=mybir.AxisListType.X, op=ALU.min)
                    nc.vector.tensor_scalar(lo[:], lo[:], -1.0, 0.0,
                                            op0=ALU.add, op1=ALU.add)
                    st.update(scratch=scratch, ekm1=ekm1, lo=lo, hi=hi,
                              mid=mid, cnt=cnt, pred=pred, predn=predn)

                def chunk(n, after_inst):
                    lo, hi, mid, cnt = st["lo"], st["hi"], st["mid"], st["cnt"]
                    pred, predn, ekm1 = st["pred"], st["predn"], st["ekm1"]
                    for j in range(n):
                        first = nc.vector.tensor_tensor(mid[:], lo[:], hi[:],
                                                        op=ALU.add)
                        if j == 0 and after_inst is not None:
                            _add_dep_helper(first.ins, after_inst.ins,
                                            sync=False,
                                            reason="search after xT copies")
                        nc.vector.tensor_scalar(mid[:], mid[:], 0.5, 0.0,
                                                op0=ALU.mult, op1=ALU.add)
                        nc.vector.tensor_scalar(
                            st["scratch"][:], logits[:], mid[0:1, 0:1], 0.0,
                            op0=ALU.is_gt, op1=ALU.add, accum_out=cnt[:])
                        nc.vector.tensor_tensor(pred[:], cnt[:], ekm1[:],
                                                op=ALU.is_gt)
                        nc.vector.tensor_tensor(predn[:], cnt[:], ekm1[:],
                                                op=ALU.is_le)
                        nc.vector.copy_predicated(lo[:], pred[:], mid[:])
                        nc.vector.copy_predicated(hi[:], predn[:], mid[:])

                def finish():
                    scratch, lo = st["scratch"], st["lo"]
                    nc.vector.tensor_scalar(
                        scratch[:], logits[:], lo[0:1, 0:1], 0.0,
                        op0=ALU.is_gt, op1=ALU.add)
                    nc.scalar.dma_start(mask_d[b:b + 1, :], scratch[:])
                    nc.scalar.dma_start(bounce[b:b + 1, :], scratch[:])
                    mc = row_pool.tile([P, X_TILES], F32, tag=f"mc{b}")
                    nc.scalar.dma_start(
                        mc[:], bounce[b, :].rearrange("(i p) -> p i", p=P))
                    return mc

                return head, chunk, finish

            # Last row: single-shot phase 2 at the tail (PE idle there).
            def make_phase2(b, logits, lcol):
                def phase2():
                    scratch = row_pool.tile([1, S], F32, tag="rowscratch")
                    ek = row_pool.tile([1, 1], F32, tag=f"ek{b}")
                    nc.scalar.activation(scratch[:], logits[:], ACT.Sigmoid,
                                         accum_out=ek[:])
                    nc.scalar.dma_start(ek_d[b:b + 1, :], ek[:])
                    # criterion: cnt >= max(floor(ek),32) <=> cnt > max(ek-1,31.5)
                    ekm1 = row_pool.tile([1, 1], F32, tag="ekm1")
                    nc.vector.tensor_scalar(ekm1[:], ek[:], -1.0, 31.5,
                                            op0=ALU.add, op1=ALU.max)

                    pe_search = (b == B_PER_CORE - 1)
                    if pe_search:
                        # token-partition logits via a burst of thin PE
                        # transposes (deps long satisfied; no queue stalls)
                        for mm in range(M_TILES):
                            ptl = psums_pool.tile([P, SUBS], F32, tag="ptl")
                            for sub in range(SUBS):
                                nc.tensor.transpose(
                                    ptl[:, sub:sub + 1],
                                    logits[:, mm * N_TILE + sub * P:
                                           mm * N_TILE + (sub + 1) * P],
                                    ones_row[0:1, 0:1])
                            nc.vector.tensor_copy(
                                lcol[:, mm * SUBS:(mm + 1) * SUBS], ptl[:])
                        lo = row_pool.tile([P, 1], F32, tag="lo128")
                        hi = row_pool.tile([P, 1], F32, tag="hi128")
                        mid = row_pool.tile([P, 1], F32, tag="mid128")
                        pc = row_pool.tile([P, 1], F32, tag="pc")
                        cscr = row_pool.tile([P, LC], F32, tag="cscr")
                        pred = row_pool.tile([P, 1], U32, tag="pred128")
                        predn = row_pool.tile([P, 1], U32, tag="predn128")
                        ekm128 = row_pool.tile([P, 1], F32, tag="ekm128")
                        ek_ps = psums_pool.tile([P, 1], F32, tag="small")
                        nc.tensor.matmul(ek_ps[:], ones_row[:], ekm1[:],
                                         start=True, stop=True)
                        nc.vector.tensor_copy(ekm128[:], ek_ps[:])
                        nc.vector.memset(lo[:], -16.0)
                        nc.vector.memset(hi[:], 16.0)
                        for _ in range(N_ITERS):
                            nc.vector.tensor_tensor(mid[:], lo[:], hi[:], op=ALU.add)
                            nc.vector.tensor_scalar(mid[:], mid[:], 0.5, 0.0,
                                                    op0=ALU.mult, op1=ALU.add)
                            nc.vector.tensor_scalar(
                                cscr[:], lcol[:], mid[:], 0.0,
                                op0=ALU.is_gt, op1=ALU.add, accum_out=pc[:])
                            cp = psums_pool.tile([P, 1], F32, tag="small")
                            nc.tensor.matmul(cp[:], ones_mat[:], pc[:],
                                             start=True, stop=True)
                            nc.vector.tensor_tensor(pred[:], cp[:], ekm128[:],
                                                    op=ALU.is_gt)
                            nc.vector.tensor_tensor(predn[:], cp[:], ekm128[:],
                                                    op=ALU.is_le)
                            nc.vector.copy_predicated(lo[:], pred[:], mid[:])
                            nc.vector.copy_predicated(hi[:], predn[:], mid[:])
                        lo128 = lo
                    else:
                        lo = row_pool.tile([1, 1], F32, tag="lo")
                        hi = row_pool.tile([1, 1], F32, tag="hi")
                        mid = row_pool.tile([1, 1], F32, tag="mid")
                        cnt = row_pool.tile([1, 1], F32, tag="cnt")
                        pred = row_pool.tile([1, 1], U32, tag="pred")
                        predn = row_pool.tile([1, 1], U32, tag="predn")
                        nc.vector.tensor_reduce(
                            hi[:], logits[:], axis=mybir.AxisListType.X, op=ALU.max)
                        nc.vector.tensor_reduce(
                            lo[:], logits[:], axis=mybir.AxisListType.X, op=ALU.min)
                        nc.vector.tensor_scalar(lo[:], lo[:], -1.0, 0.0,
                                                op0=ALU.add, op1=ALU.add)
                        for _ in range(N_ITERS):
                            nc.vector.tensor_tensor(mid[:], lo[:], hi[:], op=ALU.add)
                            nc.vector.tensor_scalar(mid[:], mid[:], 0.5, 0.0,
                                                    op0=ALU.mult, op1=ALU.add)
                            nc.vector.tensor_scalar(
                                scratch[:], logits[:], mid[0:1, 0:1], 0.0,
                                op0=ALU.is_gt, op1=ALU.add, accum_out=cnt[:])
                            nc.vector.tensor_tensor(pred[:], cnt[:], ekm1[:],
                                                    op=ALU.is_gt)
                            nc.vector.tensor_tensor(predn[:], cnt[:], ekm1[:],
                                                    op=ALU.is_le)
                            nc.vector.copy_predicated(lo[:], pred[:], mid[:])
                            nc.vector.copy_predicated(hi[:], predn[:], mid[:])
                        lo128 = lo

                    # mask row output = logits > lo
                    nc.vector.tensor_scalar(
                        scratch[:], logits[:], lo128[0:1, 0:1], 0.0,
                        op0=ALU.is_gt, op1=ALU.add)
                    nc.scalar.dma_start(mask_d[b:b + 1, :], scratch[:])
                    # mask columns for the masking pass
                    mc = row_pool.tile([P, X_TILES], F32, tag=f"mc{b}")
                    if pe_search:
                        nc.vector.tensor_scalar(
                            mc[:], lcol[:], lo128[:], 0.0,
                            op0=ALU.is_gt, op1=ALU.add)
                    else:
                        nc.scalar.dma_start(bounce[b:b + 1, :], scratch[:])
                        nc.scalar.dma_start(
                            mc[:], bounce[b, :].rearrange("(i p) -> p i", p=P))
                    return mc
                return phase2

            # ---------------- per-batch-row pipeline ----------------
            deferred_parts = None  # (head, chunk, finish) of the previous row
            deferred_phase2 = None
            pending = []
            iters_left = 0
            CHUNK = (N_ITERS + 2) // 3

            for b in range(B_PER_CORE):
                logits = row_pool.tile([1, S], F32, tag=f"logits{b}")
                lcol = row_pool.tile([P, LC], F32, tag=f"lcol{b}")

                for m in range(M_TILES):
                    tok0 = b * S + m * N_TILE
                    # ---- load + transpose x macro tile -> xT[d] (f32r)
                    stg = [[None] * DHALVES for _ in range(SUBS)]
                    for sub in range(SUBS):
                        for hf in range(DHALVES):
                            st = stage_pool.tile([P, HALF], F32, tag="stage")
                            nc.sync.dma_start(
                                st[:],
                                x_d[tok0 + sub * P: tok0 + (sub + 1) * P,
                                    hf * HALF:(hf + 1) * HALF])
                            stg[sub][hf] = st
                    xts = []
                    for d in range(DC):
                        hf, off = divmod(d * P, HALF)
                        pt = psumt_pool.tile([P, N_TILE], F32, tag="pt")
                        for sub in range(SUBS):
                            nc.tensor.transpose(
                                pt[:, sub * P:(sub + 1) * P],
                                stg[sub][hf][:, off:off + P], ident[:])
                        xt = xt_pool.tile([P, N_TILE], F32R, tag=f"xt{d}")
                        last_xt = nc.vector.tensor_copy(xt[:], pt[:])
                        xts.append(xt)

                    # ---- GEMM1 (f32r) + relu-evict (ACT) + GEMM2 (fp32)
                    pl = psuml_pool.tile([1, N_TILE], F32, tag="pl")
                    for h in range(HC):
                        ph = psum_pool.tile([P, N_TILE], F32, tag="ph")
                        for d in range(DC):
                            nc.tensor.matmul(
                                ph[:], w1_t[d][:, h * P:(h + 1) * P], xts[d][:],
                                start=(d == 0), stop=(d == DC - 1))
                        ht = ht_pool.tile([P, N_TILE], F32, tag="ht")
                        nc.scalar.activation(ht[:], ph[:], ACT.Relu,
                                             bias=b1_sb[:, h:h + 1])
                        nc.tensor.matmul(
                            pl[:], w2_sb[:, h:h + 1], ht[:],
                            start=(h == 0), stop=(h == HC - 1),
                            skip_group_check=True)
                    # logits row slice = psum + b2
                    nc.vector.tensor_scalar(
                        logits[:, m * N_TILE:(m + 1) * N_TILE], pl[0:1, :],
                        b2_sb[0:1, 0:1], 0.0, op0=ALU.add, op1=ALU.add)
                    # weave the previous row's phase 2 into this GEMM stream
                    if deferred_parts is not None:
                        p2_head, p2_chunk, p2_finish = deferred_parts
                        if m == 0:
                            p2_head(last_xt)
                            p2_chunk(CHUNK, None)
                            iters_left = N_ITERS - CHUNK
                        elif iters_left > 0:
                            take = min(CHUNK, iters_left)
                            p2_chunk(take, last_xt)
                            iters_left -= take
                            if iters_left == 0:
                                mc_prev = p2_finish()
                                pending = list(mask_pass_units(
                                    b - 1, mc_prev, False))
                                deferred_parts = None
                    elif pending:
                        # spread the previous row's masking pass over macros
                        for _ in range(7):
                            if pending:
                                pending.pop(0)()

                if deferred_parts is not None:
                    # small-config fallback: flush the previous row's phase 2
                    p2_head, p2_chunk, p2_finish = deferred_parts
                    if iters_left >= N_ITERS:
                        p2_head(None)
                    if iters_left > 0:
                        p2_chunk(iters_left, None)
                    mc_prev = p2_finish()
                    pending = list(mask_pass_units(b - 1, mc_prev, False))
                    deferred_parts = None
                while pending:
                    pending.pop(0)()
                if b < B_PER_CORE - 1:
                    deferred_parts = make_phase2_parts(b, logits)
                    iters_left = N_ITERS
                else:
                    deferred_phase2 = make_phase2(b, logits, lcol)

            # tail: last row's phase 2 + masking pass on both queue families
            mc_last = deferred_phase2()
            for unit in mask_pass_units(B_PER_CORE - 1, mc_last, True):
                unit()

    nc.compile()
    return nc


_FULL_NC = None
TRACE = False          # test harness sets True to capture exec_time_ns
LAST_EXEC_NS = None
LAST_RESULT = None


def kernel(token_embeddings, W1, b1, W2, b2):
    global _FULL_NC
    B, S, D = token_embeddings.shape
    H = W1.shape[1]
    assert (B, S, D, H) == (16, 2048, 2048, 2048), (B, S, D, H)
    bpc = B // N_CORES
    if _FULL_NC is None:
        _FULL_NC = build(B_PER_CORE=bpc, S=S, D=D, H=H)
    nc = _FULL_NC

    x = np.ascontiguousarray(np.asarray(token_embeddings, dtype=np.float32))
    W1 = np.ascontiguousarray(np.asarray(W1, dtype=np.float32))
    b1 = np.ascontiguousarray(np.asarray(b1, dtype=np.float32))
    W2 = np.ascontiguousarray(np.asarray(W2, dtype=np.float32))
    b2 = np.ascontiguousarray(np.asarray(b2, dtype=np.float32)).reshape(1, 1)

    in_maps = []
    for c in range(N_CORES):
        in_maps.append({
            "x": x[c * bpc:(c + 1) * bpc].reshape(bpc * S, D),
            "W1": W1, "b1": b1, "W2": W2, "b2": b2,
        })
    global LAST_EXEC_NS, LAST_RESULT
    res = run_bass_kernel_spmd(nc, in_maps, core_ids=list(range(N_CORES)),
                               trace=TRACE)
    LAST_EXEC_NS = res.exec_time_ns
    LAST_RESULT = res
    filtered = np.empty((B, S, D), np.float32)
    mask = np.empty((B, S), np.float32)
    ek = np.empty((B,), np.float32)
    for c, r in enumerate(res.results):
        filtered[c * bpc:(c + 1) * bpc] = r["filtered"].reshape(bpc, S, D)
        mask[c * bpc:(c + 1) * bpc] = r["mask"]
        ek[c * bpc:(c + 1) * bpc] = r["ek"].ravel()
    return filtered, mask, ek
